# revision 30
# baseline (speedup 1.0000x reference)
"""Multi-head self-attention (b=2, n=2048, dim=1024, H=16, D=64) on 8 trn2 NeuronCores.

Sharding: tensor-parallel over heads (4 groups of 4 heads) x data-parallel over
batch (2). Core c handles batch c//4, head group c%4. Each core computes its
head group's QKV projection, RoPE, attention, and a partial output projection;
the host sums the 4 per-group partials per batch (the "all-reduce") and adds
b_out.

v2: all-bf16 datapath (inputs pre-cast on host). Rationale:
  - bf16 matmuls stream 1 col/cycle like fp32r but LDWEIGHTS runs 2 els/cycle
    (FWL) and is a separate instruction the PE queue can pull ahead.
  - score matmuls for the two heads of a pair live at partition rows 0-63 /
    64-127 -> auto tile_position (0,0)/(64,0) row-groups, which the PE runs
    concurrently (K=64 each).
  - end-to-end bf16 rel-err ~5e-3, well under the 2e-2 gate.
Schedule: attention (scores -> exp on ACT -> AV into PSUM accumulators) is the
ACT-bound steady state; all projection / rope / out-projection PE work is fed
through a filler queue drained one unit per k-tile so the PE never idles while
waiting on exp, and segment-boundary stalls (PSUM accumulator reuse vs the
softmax-normalize chain) are covered by the same filler.
PSUM budget (8 banks): scores ring 2x[128,1024] (4) + AV accumulators
2x[65,512] (2) + proj/rot/out-proj ring 2x[128,512] (2).
"""

import numpy as np
import ml_dtypes

import concourse.bass as bass
import concourse.mybir as mybir
import concourse.tile as tile
from concourse.tile_rust import add_dep_helper
from concourse import bacc
from concourse.bass_utils import run_bass_kernel_spmd

BF = mybir.dt.bfloat16
F32 = mybir.dt.float32

# Full-problem constants
B, N_SEQ, DIM, H, D = 2, 2048, 1024, 16, 64
TP = 4                      # head-group parallel degree
HPC = H // TP               # heads per core = 4
N_CORES = 8
NPDT = ml_dtypes.bfloat16


class Cfg:
    def __init__(self, n_seq=N_SEQ, dim=DIM):
        self.n_seq = n_seq
        self.dim = dim
        self.dt = dim // 128          # contraction dim tiles
        self.kt = n_seq // 128        # k tiles
        self.qc2 = n_seq // 1024      # 1024-wide q chunks
        self.fpc = HPC * D            # features per core (q or k or v) = 256


HEAD_MODE = 0               # 0 seq | 1 proj x v | 2 attn x proj | 3 full (1-3 corrupt on HW)
STEADY_DRAIN = True         # drain filler units inside steady segments


def build_nc(cfg: Cfg, repeat: int = 1):
    """Build the per-core Bass program. repeat>1 wraps the whole computation in
    a hardware For_i loop (timing harness only — output is idempotent)."""
    import contextlib
    nc = bacc.Bacc()
    n, dim, DT, KT = cfg.n_seq, cfg.dim, cfg.dt, cfg.kt
    NCH = n // 512                  # 512-wide n/q chunks
    QC = NCH

    xT = nc.dram_tensor("xT", [dim, n], BF, kind="ExternalInput")
    wqk = nc.dram_tensor("wqk", [dim, 2 * cfg.fpc], BF, kind="ExternalInput")
    wv = nc.dram_tensor("wv", [dim, cfg.fpc], BF, kind="ExternalInput")
    wo = nc.dram_tensor("wo", [cfg.fpc, dim], BF, kind="ExternalInput")
    cosT = nc.dram_tensor("cosT", [128, n], BF, kind="ExternalInput")
    sinT = nc.dram_tensor("sinT", [128, n], BF, kind="ExternalInput")
    srot = nc.dram_tensor("srot", [128, 128], BF, kind="ExternalInput")
    onesv = nc.dram_tensor("onesv", [128, KT * HPC], BF, kind="ExternalInput")
    out = nc.dram_tensor("out", [n, dim], F32, kind="ExternalOutput")

    with tile.TileContext(nc) as tc:
        with (
            tc.tile_pool(name="persist", bufs=1) as persist,
            tc.tile_pool(name="qkv_sb", bufs=1) as qsb,
            tc.tile_pool(name="qkv_work", bufs=2) as qwork,
            tc.tile_pool(name="at_p", bufs=3) as p_pool,
            tc.tile_pool(name="at_o2", bufs=2) as o2_pool,
            tc.tile_pool(name="at_small", bufs=2) as small,
            tc.tile_pool(name="at_out", bufs=3) as outp,
            tc.tile_pool(name="ps_qp", bufs=2, space="PSUM") as qps,
            tc.tile_pool(name="ps_s", bufs=2, space="PSUM") as sps,
            tc.tile_pool(name="ps_po", bufs=1, space="PSUM") as pops,
        ):
          loop_ctx = tc.For_i(0, repeat, 1) if repeat > 1 else contextlib.nullcontext()
          with loop_ctx:
            # persistent SBUF
            wo_sb = [persist.tile([128, dim], BF, tag=f"wo{i}", name=f"wo{i}") for i in range(2)]
            # qkT[0],[1]: roped qT for head pairs 0,1; [2],[3]: roped kT
            qkT = [persist.tile([128, n], BF, tag=f"qkT{i}", name=f"qkT{i}") for i in range(4)]
            v_ext = persist.tile([128, KT, HPC, 65], BF, tag="vext", name="v_ext")
            srot_sb = persist.tile([128, 128], BF, tag="srot", name="srot_sb")

            xT_sb = [qsb.tile([128, n], BF, tag=f"xt{d_}", name=f"xt{d_}") for d_ in range(DT)]
            wqk_sb = [qsb.tile([128, 2 * cfg.fpc], BF, tag=f"wqk{d_}", name=f"wqk{d_}") for d_ in range(DT)]
            wv_sb = [qsb.tile([128, cfg.fpc], BF, tag=f"wv{d_}", name=f"wv{d_}") for d_ in range(DT)]
            # critical-path loads first: wqk + x chunk 0 + srot + cos/sin feed
            # the first projections; remaining x chunks and wv stream after.
            for d_ in range(DT):
                nc.sync.dma_start(out=wqk_sb[d_], in_=wqk[d_ * 128:(d_ + 1) * 128, :])
                nc.sync.dma_start(
                    out=xT_sb[d_][:, 0:512],
                    in_=xT[d_ * 128:(d_ + 1) * 128, 0:512],
                )
            nc.sync.dma_start(out=srot_sb, in_=srot[:, :])
            cos_sb = qsb.tile([128, n], BF, tag="cos", name="cos_sb")
            sin_sb = qsb.tile([128, n], BF, tag="sin", name="sin_sb")
            nc.sync.dma_start(out=cos_sb, in_=cosT[:, :])
            nc.sync.dma_start(out=sin_sb, in_=sinT[:, :])
            for d_ in range(DT):
                nc.sync.dma_start(out=wv_sb[d_], in_=wv[d_ * 128:(d_ + 1) * 128, :])
                for c in range(1, NCH):
                    nc.sync.dma_start(
                        out=xT_sb[d_][:, c * 512:(c + 1) * 512],
                        in_=xT[d_ * 128:(d_ + 1) * 128, c * 512:(c + 1) * 512],
                    )
            for i in range(2):
                nc.sync.dma_start(out=wo_sb[i], in_=wo[i * 128:(i + 1) * 128, :])
            nc.sync.dma_start(
                out=v_ext[:, :, :, 64:65],
                in_=onesv[:, :].rearrange("p (k h o) -> p k h o", h=HPC, o=1),
            )

            qk_write, v_write = {}, {}

            def proj_half(ft, c, half, ps):
                csl = slice(c * 512, (c + 1) * 512)
                for d_ in range(4 * half, 4 * half + 4):
                    nc.tensor.matmul(
                        ps,
                        wqk_sb[d_][:, ft * 128:(ft + 1) * 128],
                        xT_sb[d_][:, csl],
                        start=(d_ == 0),
                        stop=(d_ == DT - 1),
                    )
                if half == 0:
                    return
                pre = qwork.tile([128, 512], BF, tag="pre", name="pre")
                nc.vector.tensor_copy(pre, ps)
                rot = qps.tile([128, 512], F32, tag="qp", name="ps_rot")
                nc.tensor.matmul(rot, srot_sb, pre, start=True, stop=True)
                dst = qkT[ft][:, csl]
                nc.vector.tensor_mul(dst, pre, cos_sb[:, csl])
                t2 = qwork.tile([128, 512], BF, tag="t2", name="t2")
                nc.vector.tensor_mul(t2, rot, sin_sb[:, csl])
                qk_write[(ft, c)] = nc.vector.tensor_add(dst, dst, t2)

            def proj_chunk(ft, c):
                """project w_qkv feature tile ft for n-chunk c, apply rope into qkT[ft]."""
                ps = qps.tile([128, 512], F32, tag="qp", name="ps_qk")
                proj_half(ft, c, 0, ps)
                proj_half(ft, c, 1, ps)

            def v_chunk(kt):
                psv = qps.tile([128, cfg.fpc], F32, tag="qp", name="ps_v")
                for d_ in range(DT):
                    nc.tensor.matmul(
                        psv,
                        xT_sb[d_][:, kt * 128:(kt + 1) * 128],
                        wv_sb[d_],
                        start=(d_ == 0),
                        stop=(d_ == DT - 1),
                    )
                v_write[kt] = nc.vector.tensor_copy(
                    v_ext[:, kt, :, 0:64],
                    psv.rearrange("p (h d) -> p h d", h=HPC),
                )

            # ---- filler queue: PE work fed into the attention stream ----
            # Emission order IS dependency order (Tile traces program order):
            # every filler unit is emitted at least one full segment before
            # anything reads the tiles it writes (fresh-stationary writes
            # consumed sooner than that have been observed to corrupt on HW).
            work = []
            consumed = [0]

            def drain(k):
                for _ in range(min(k, len(work))):
                    work.pop(0)()
                    consumed[0] += 1

            def drain_to(k):
                drain(max(0, k - consumed[0]))

            def emit_scores(sidx, kt):
                qc, hp = SEGS[sidx]
                qsl = slice(qc * 512, (qc + 1) * 512)
                ksl = slice(kt * 128, (kt + 1) * 128)
                # two K=64 matmuls on PE row-groups (0,0)/(64,0): concurrent
                ps_s = sps.tile([128, 1024], F32, tag="s", name="ps_s")
                for hh in range(2):
                    psl = slice(64 * hh, 64 * (hh + 1))
                    nc.tensor.matmul(
                        ps_s[:, hh * 512:(hh + 1) * 512],
                        qkT[2 + hp][psl, ksl],
                        qkT[hp][psl, qsl],
                        start=True,
                        stop=True,
                    )
                return ps_s

            def emit_exp(sidx, kt, ps_s):
                p_sb = p_pool.tile([128, 1024], BF, tag="p", name="p_sb")
                nc.scalar.activation(
                    p_sb, ps_s, mybir.ActivationFunctionType.Exp,
                    scale=float(1.0 / np.sqrt(D)),
                )
                p_of[(sidx, kt)] = p_sb

            def emit_av(sidx, kt):
                qc, hp = SEGS[sidx]
                p_sb = p_of.pop((sidx, kt))
                # AV with ones column: row 64 accumulates the denominator
                for hh in range(2):
                    nc.tensor.matmul(
                        po_of[sidx][hh],
                        v_ext[:, kt, 2 * hp + hh, :],
                        p_sb[:, hh * 512:(hh + 1) * 512],
                        start=(kt == 0),
                        stop=(kt == KT - 1),
                    )

            def norm_pair(sidx):
                # copy the accumulators out of PSUM first: releases the po
                # banks fast; the divide/broadcast/mul chain then runs on the
                # SBUF copies off the critical path.
                po = po_of.pop(sidx)
                o2 = o2_pool.tile([128, 512], BF, tag="o2", name="o2")
                poc = [o2_pool.tile([65, 512], F32, tag=f"poc{hh}", name="poc") for hh in range(2)]
                for hh in range(2):
                    nc.vector.tensor_copy(poc[hh], po[hh])
                for hh in range(2):
                    rrec = small.tile([1, 512], F32, tag="rrec", name="rrec")
                    nc.vector.reciprocal(rrec, poc[hh][64:65, :])
                    bc = small.tile([64, 512], F32, tag="bc", name="bc")
                    nc.gpsimd.partition_broadcast(bc, rrec)
                    nc.vector.tensor_mul(o2[64 * hh:64 * (hh + 1), :], poc[hh][0:64, :], bc)
                return o2

            def outproj_unit(qc, qt, o2l):
                row = (qc * 4 + qt) * 128
                pso = sps.tile([128, 1024], F32, tag="s", name="pso")
                for od in range(2):
                    osl = slice(od * 512, (od + 1) * 512)
                    for hp in range(2):
                        nc.tensor.matmul(
                            pso[:, osl],
                            o2l[hp][:, qt * 128:(qt + 1) * 128],
                            wo_sb[hp][:, osl],
                            start=(hp == 0),
                            stop=(hp == 1),
                        )
                ob = outp.tile([128, 1024], F32, tag="ob", name="ob")
                nc.vector.tensor_copy(ob, pso)
                nc.sync.dma_start(out=out[row:row + 128, :], in_=ob)

            # segments in (qc, hp) order; stream(s) runs scores+exp of segment
            # s while the PE retires AV of segment s-1 (p tiles buffer a full
            # segment). The last stream is coupled (AV follows its own exp) so
            # the tail is just normalize + the final out-projection.
            SEGS = [(qc, hp) for qc in range(QC) for hp in range(2)]
            NSEG = len(SEGS)
            p_of, po_of = {}, {}
            o2_all = {}

            # projection filler units, in first-needed order; stream(s) drains
            # the units segment s+1 needs, one full segment ahead of use.
            pending_ps = {}

            def proj_unit(ft, c, half):
                if half == 0:
                    pending_ps[(ft, c)] = qps.tile([128, 512], F32, tag="qp", name="ps_qk")
                proj_half(ft, c, half, pending_ps[(ft, c)])
                if half == 1:
                    del pending_ps[(ft, c)]

            chunk_order = ([(3, c) for c in range(NCH)] + [(1, 0)]
                           + [(f, c) for c in range(1, NCH) for f in (0, 1)])
            for ft, c in chunk_order:
                for half in range(2):
                    work.append(lambda ft=ft, c=c, h=half: proj_unit(ft, c, h))
            prereq = {1: 10}
            for s in range(2, NSEG):
                prereq[s] = 8 + 2 * s

            # head: k/q projections for segment 0 (sequential: only settled
            # stationaries are ever consumed)
            proj_chunk(2, 0)
            proj_chunk(0, 0)
            for c in range(1, NCH):
                proj_chunk(2, c)

            pend = emit_scores(0, 0)
            for sidx in range(NSEG):
                qc, hp = SEGS[sidx]
                coupled = (sidx == NSEG - 1)
                if sidx > 0:
                    # AV of the previous segment retires during this stream
                    po_of[sidx - 1] = [pops.tile([65, 512], F32, tag=f"po{hh}", name=f"po{hh}")
                                       for hh in range(2)]
                if coupled:
                    # last stream also retires its own AV; its accumulators
                    # live in the (now idle) qp ring banks
                    po_of[sidx] = [qps.tile([65, 512], F32, tag="qp", name=f"poz{hh}")
                                   for hh in range(2)]
                for i in range(KT):
                    ps_s = pend
                    nxt = (sidx, i + 1) if i + 1 < KT else (
                        (sidx + 1, 0) if sidx + 1 < NSEG else None)
                    if nxt is not None:
                        if nxt[0] != sidx and nxt[0] in prereq:
                            drain_to(prereq[nxt[0]])
                        pend = emit_scores(nxt[0], nxt[1])
                    emit_exp(sidx, i, ps_s)
                    if sidx == 0:
                        v_chunk(i)
                    else:
                        emit_av(sidx - 1, i)
                    if coupled:
                        emit_av(sidx, i)
                    elif i % 2 == 1 and i + 2 < KT:
                        drain(1)

                if sidx > 0:
                    s_done = sidx - 1
                    o2_all[s_done] = norm_pair(s_done)
                    qcd, hpd = SEGS[s_done]
                    if hpd == 1:
                        o2l = (o2_all.pop(2 * qcd), o2_all.pop(2 * qcd + 1))
                        for qt in range(4):
                            work.append(lambda qt=qt, qc=qcd, o2l=o2l: outproj_unit(qc, qt, o2l))
                if coupled:
                    o2_all[sidx] = norm_pair(sidx)
                    o2l = (o2_all.pop(sidx - 1), o2_all.pop(sidx))
                    for qt in range(4):
                        work.append(lambda qt=qt, qc=qc, o2l=o2l: outproj_unit(qc, qt, o2l))
            drain(len(work))

    nc.finalize()
    return nc


def rope_tables(n, d):
    """cos/sin tables in (d, n) layout, interleaved-repeat, theta=10000."""
    inv_freq = 1.0 / (10000.0 ** (np.arange(0, d, 2, dtype=np.float32) / d))
    ang = np.arange(n, dtype=np.float32)[:, None] * inv_freq[None, :]   # (n, d/2)
    cos = np.repeat(np.cos(ang), 2, axis=-1).T.copy()                    # (d, n)
    sin = np.repeat(np.sin(ang), 2, axis=-1).T.copy()
    return cos.astype(np.float32), sin.astype(np.float32)


def rot_matrix(d):
    """S with (S x)[2i] = -x[2i+1], (S x)[2i+1] = x[2i]."""
    S = np.zeros((d, d), dtype=np.float32)
    for i in range(d // 2):
        S[2 * i, 2 * i + 1] = -1.0
        S[2 * i + 1, 2 * i] = 1.0
    return S


def make_core_inputs(x, w_qkv, w_out, cfg: Cfg, core):
    n, dim = cfg.n_seq, cfg.dim
    b, g = core // TP, core % TP
    f0 = g * cfg.fpc
    inner = TP * cfg.fpc
    xT = np.ascontiguousarray(np.asarray(x)[b].T).astype(NPDT)
    w_qkv = np.asarray(w_qkv)
    wq = w_qkv[:, f0:f0 + cfg.fpc]
    wk = w_qkv[:, inner + f0:inner + f0 + cfg.fpc]
    wvv = np.ascontiguousarray(w_qkv[:, 2 * inner + f0:2 * inner + f0 + cfg.fpc]).astype(NPDT)
    wqk = np.ascontiguousarray(np.concatenate([wq, wk], axis=1)).astype(NPDT)
    wo = np.ascontiguousarray(np.asarray(w_out)[f0:f0 + cfg.fpc, :]).astype(NPDT)
    cos, sin = rope_tables(n, D)
    cosT = np.concatenate([cos, cos], axis=0).astype(NPDT)   # 2-head packed (128, n)
    sinT = np.concatenate([sin, sin], axis=0).astype(NPDT)
    S = rot_matrix(D)
    S128 = np.zeros((128, 128), dtype=np.float32)
    S128[0:64, 0:64] = S
    S128[64:128, 64:128] = S
    srot = np.ascontiguousarray(S128.T).astype(NPDT)
    onesv = np.ones((128, cfg.kt * HPC), dtype=NPDT)
    return {
        "xT": xT, "wqk": wqk, "wv": wvv, "wo": wo,
        "cosT": cosT, "sinT": sinT, "srot": srot, "onesv": onesv,
    }


_NC_CACHE = {}


def kernel(x, w_qkv, w_out, b_out):
    cfg = Cfg()
    key = (cfg.n_seq, cfg.dim)
    if key not in _NC_CACHE:
        _NC_CACHE[key] = build_nc(cfg)
    nc = _NC_CACHE[key]
    in_maps = [make_core_inputs(x, w_qkv, w_out, cfg, c) for c in range(N_CORES)]
    res = run_bass_kernel_spmd(nc, in_maps, core_ids=list(range(N_CORES)))
    partials = [r["out"] for r in res.results]
    out = np.empty((B, cfg.n_seq, cfg.dim), dtype=np.float32)
    for b in range(B):
        acc = partials[b * TP].astype(np.float32).copy()
        for g in range(1, TP):
            acc += partials[b * TP + g]
        out[b] = acc + np.asarray(b_out, dtype=np.float32)[None, :]
    return out


# revision 31
# speedup vs baseline: 1.2279x; 1.2279x over previous
"""Multi-head self-attention (b=2, n=2048, dim=1024, H=16, D=64) on 8 trn2 NeuronCores.

Sharding: tensor-parallel over heads (4 groups of 4 heads) x data-parallel over
batch (2). Core c handles batch c//4, head group c%4. Each core computes its
head group's QKV projection, RoPE, attention, and a partial output projection;
the host sums the 4 per-group partials per batch (the "all-reduce") and adds
b_out.

v2: all-bf16 datapath (inputs pre-cast on host). Rationale:
  - bf16 matmuls stream 1 col/cycle like fp32r but LDWEIGHTS runs 2 els/cycle
    (FWL) and is a separate instruction the PE queue can pull ahead.
  - score matmuls for the two heads of a pair live at partition rows 0-63 /
    64-127 -> auto tile_position (0,0)/(64,0) row-groups, which the PE runs
    concurrently (K=64 each).
  - end-to-end bf16 rel-err ~5e-3, well under the 2e-2 gate.
Schedule: attention (scores -> exp on ACT -> AV into PSUM accumulators) is the
ACT-bound steady state; all projection / rope / out-projection PE work is fed
through a filler queue drained one unit per k-tile so the PE never idles while
waiting on exp, and segment-boundary stalls (PSUM accumulator reuse vs the
softmax-normalize chain) are covered by the same filler.
PSUM budget (8 banks): scores ring 2x[128,1024] (4) + AV accumulators
2x[65,512] (2) + proj/rot/out-proj ring 2x[128,512] (2).
"""

import numpy as np
import ml_dtypes

import concourse.bass as bass
import concourse.mybir as mybir
import concourse.tile as tile
from concourse.tile_rust import add_dep_helper
from concourse import bacc
from concourse.bass_utils import run_bass_kernel_spmd

BF = mybir.dt.bfloat16
F32 = mybir.dt.float32

# Full-problem constants
B, N_SEQ, DIM, H, D = 2, 2048, 1024, 16, 64
TP = 4                      # head-group parallel degree
HPC = H // TP               # heads per core = 4
N_CORES = 8
NPDT = ml_dtypes.bfloat16


class Cfg:
    def __init__(self, n_seq=N_SEQ, dim=DIM):
        self.n_seq = n_seq
        self.dim = dim
        self.dt = dim // 128          # contraction dim tiles
        self.kt = n_seq // 128        # k tiles
        self.qc2 = n_seq // 1024      # 1024-wide q chunks
        self.fpc = HPC * D            # features per core (q or k or v) = 256


HEAD_MODE = 0               # 0 seq | 1 proj x v | 2 attn x proj | 3 full (1-3 corrupt on HW)
STEADY_DRAIN = True         # drain filler units inside steady segments


def build_nc(cfg: Cfg, repeat: int = 1):
    """Build the per-core Bass program. repeat>1 wraps the whole computation in
    a hardware For_i loop (timing harness only — output is idempotent)."""
    import contextlib
    nc = bacc.Bacc()
    n, dim, DT, KT = cfg.n_seq, cfg.dim, cfg.dt, cfg.kt
    NCH = n // 512                  # 512-wide n/q chunks
    QC = NCH

    xT = nc.dram_tensor("xT", [dim, n], BF, kind="ExternalInput")
    wqk = nc.dram_tensor("wqk", [dim, 2 * cfg.fpc], BF, kind="ExternalInput")
    wv = nc.dram_tensor("wv", [dim, cfg.fpc], BF, kind="ExternalInput")
    wo = nc.dram_tensor("wo", [cfg.fpc, dim], BF, kind="ExternalInput")
    cosT = nc.dram_tensor("cosT", [128, n], BF, kind="ExternalInput")
    sinT = nc.dram_tensor("sinT", [128, n], BF, kind="ExternalInput")
    srot = nc.dram_tensor("srot", [128, 128], BF, kind="ExternalInput")
    onesv = nc.dram_tensor("onesv", [128, KT * HPC], BF, kind="ExternalInput")
    out = nc.dram_tensor("out", [n, dim], F32, kind="ExternalOutput")

    with tile.TileContext(nc) as tc:
        with (
            tc.tile_pool(name="persist", bufs=1) as persist,
            tc.tile_pool(name="qkv_sb", bufs=1) as qsb,
            tc.tile_pool(name="qkv_work", bufs=2) as qwork,
            tc.tile_pool(name="at_p", bufs=3) as p_pool,
            tc.tile_pool(name="at_o2", bufs=2) as o2_pool,
            tc.tile_pool(name="at_small", bufs=2) as small,
            tc.tile_pool(name="at_out", bufs=3) as outp,
            tc.tile_pool(name="ps_qp", bufs=2, space="PSUM") as qps,
            tc.tile_pool(name="ps_s", bufs=2, space="PSUM") as sps,
            tc.tile_pool(name="ps_po", bufs=1, space="PSUM") as pops,
        ):
          loop_ctx = tc.For_i(0, repeat, 1) if repeat > 1 else contextlib.nullcontext()
          with loop_ctx:
            # persistent SBUF
            wo_sb = [persist.tile([128, dim], BF, tag=f"wo{i}", name=f"wo{i}") for i in range(2)]
            # qkT[0],[1]: roped qT for head pairs 0,1; [2],[3]: roped kT
            qkT = [persist.tile([128, n], BF, tag=f"qkT{i}", name=f"qkT{i}") for i in range(4)]
            v_ext = persist.tile([128, KT, HPC, 65], BF, tag="vext", name="v_ext")
            srot_sb = persist.tile([128, 128], BF, tag="srot", name="srot_sb")

            xT_sb = [qsb.tile([128, n], BF, tag=f"xt{d_}", name=f"xt{d_}") for d_ in range(DT)]
            wqk_sb = [qsb.tile([128, 2 * cfg.fpc], BF, tag=f"wqk{d_}", name=f"wqk{d_}") for d_ in range(DT)]
            wv_sb = [qsb.tile([128, cfg.fpc], BF, tag=f"wv{d_}", name=f"wv{d_}") for d_ in range(DT)]
            # critical-path loads first: wqk + x chunk 0 + srot + cos/sin feed
            # the first projections; remaining x chunks and wv stream after.
            for d_ in range(DT):
                nc.sync.dma_start(out=wqk_sb[d_], in_=wqk[d_ * 128:(d_ + 1) * 128, :])
                nc.sync.dma_start(
                    out=xT_sb[d_][:, 0:512],
                    in_=xT[d_ * 128:(d_ + 1) * 128, 0:512],
                )
            nc.sync.dma_start(out=srot_sb, in_=srot[:, :])
            cos_sb = qsb.tile([128, n], BF, tag="cos", name="cos_sb")
            sin_sb = qsb.tile([128, n], BF, tag="sin", name="sin_sb")
            nc.sync.dma_start(out=cos_sb, in_=cosT[:, :])
            nc.sync.dma_start(out=sin_sb, in_=sinT[:, :])
            for d_ in range(DT):
                nc.sync.dma_start(out=wv_sb[d_], in_=wv[d_ * 128:(d_ + 1) * 128, :])
                for c in range(1, NCH):
                    nc.sync.dma_start(
                        out=xT_sb[d_][:, c * 512:(c + 1) * 512],
                        in_=xT[d_ * 128:(d_ + 1) * 128, c * 512:(c + 1) * 512],
                    )
            for i in range(2):
                nc.sync.dma_start(out=wo_sb[i], in_=wo[i * 128:(i + 1) * 128, :])
            nc.sync.dma_start(
                out=v_ext[:, :, :, 64:65],
                in_=onesv[:, :].rearrange("p (k h o) -> p k h o", h=HPC, o=1),
            )

            qk_write, v_write = {}, {}

            def proj_half(ft, c, half, ps):
                csl = slice(c * 512, (c + 1) * 512)
                for d_ in range(4 * half, 4 * half + 4):
                    nc.tensor.matmul(
                        ps,
                        wqk_sb[d_][:, ft * 128:(ft + 1) * 128],
                        xT_sb[d_][:, csl],
                        start=(d_ == 0),
                        stop=(d_ == DT - 1),
                    )
                if half == 0:
                    return
                pre = qwork.tile([128, 512], BF, tag="pre", name="pre")
                nc.vector.tensor_copy(pre, ps)
                rot = qps.tile([128, 512], F32, tag="qp", name="ps_rot")
                nc.tensor.matmul(rot, srot_sb, pre, start=True, stop=True)
                dst = qkT[ft][:, csl]
                nc.vector.tensor_mul(dst, pre, cos_sb[:, csl])
                t2 = qwork.tile([128, 512], BF, tag="t2", name="t2")
                nc.vector.tensor_mul(t2, rot, sin_sb[:, csl])
                qk_write[(ft, c)] = nc.vector.tensor_add(dst, dst, t2)

            def proj_chunk(ft, c):
                """project w_qkv feature tile ft for n-chunk c, apply rope into qkT[ft]."""
                ps = qps.tile([128, 512], F32, tag="qp", name="ps_qk")
                proj_half(ft, c, 0, ps)
                proj_half(ft, c, 1, ps)

            def v_chunk(kt):
                psv = qps.tile([128, cfg.fpc], F32, tag="qp", name="ps_v")
                for d_ in range(DT):
                    nc.tensor.matmul(
                        psv,
                        xT_sb[d_][:, kt * 128:(kt + 1) * 128],
                        wv_sb[d_],
                        start=(d_ == 0),
                        stop=(d_ == DT - 1),
                    )
                v_write[kt] = nc.vector.tensor_copy(
                    v_ext[:, kt, :, 0:64],
                    psv.rearrange("p (h d) -> p h d", h=HPC),
                )

            # ---- filler queue: PE work fed into the attention stream ----
            # Emission order IS dependency order (Tile traces program order):
            # every filler unit is emitted at least one full segment before
            # anything reads the tiles it writes (fresh-stationary writes
            # consumed sooner than that have been observed to corrupt on HW).
            work = []
            consumed = [0]

            def drain(k):
                for _ in range(min(k, len(work))):
                    work.pop(0)()
                    consumed[0] += 1

            def drain_to(k):
                drain(max(0, k - consumed[0]))

            def emit_scores(sidx, kt):
                qc, hp = SEGS[sidx]
                qsl = slice(qc * 512, (qc + 1) * 512)
                ksl = slice(kt * 128, (kt + 1) * 128)
                # two K=64 matmuls on PE row-groups (0,0)/(64,0): concurrent
                ps_s = sps.tile([128, 1024], F32, tag="s", name="ps_s")
                for hh in range(2):
                    psl = slice(64 * hh, 64 * (hh + 1))
                    nc.tensor.matmul(
                        ps_s[:, hh * 512:(hh + 1) * 512],
                        qkT[2 + hp][psl, ksl],
                        qkT[hp][psl, qsl],
                        start=True,
                        stop=True,
                    )
                return ps_s

            def emit_exp(sidx, kt, ps_s):
                p_sb = p_pool.tile([128, 1024], BF, tag="p", name="p_sb")
                nc.scalar.activation(
                    p_sb, ps_s, mybir.ActivationFunctionType.Exp,
                    scale=float(1.0 / np.sqrt(D)),
                )
                p_of[(sidx, kt)] = p_sb

            def emit_av(sidx, kt):
                qc, hp = SEGS[sidx]
                p_sb = p_of.pop((sidx, kt))
                # AV with ones column: row 64 accumulates the denominator
                for hh in range(2):
                    nc.tensor.matmul(
                        po_of[sidx][hh],
                        v_ext[:, kt, 2 * hp + hh, :],
                        p_sb[:, hh * 512:(hh + 1) * 512],
                        start=(kt == 0),
                        stop=(kt == KT - 1),
                    )

            def norm_pair(sidx):
                # copy the accumulators out of PSUM first: releases the po
                # banks fast; the divide/broadcast/mul chain then runs on the
                # SBUF copies off the critical path.
                po = po_of.pop(sidx)
                o2 = o2_pool.tile([128, 512], BF, tag="o2", name="o2")
                poc = [o2_pool.tile([65, 512], F32, tag=f"poc{hh}", name="poc") for hh in range(2)]
                for hh in range(2):
                    nc.vector.tensor_copy(poc[hh], po[hh])
                for hh in range(2):
                    rrec = small.tile([1, 512], F32, tag="rrec", name="rrec")
                    nc.vector.reciprocal(rrec, poc[hh][64:65, :])
                    bc = small.tile([64, 512], F32, tag="bc", name="bc")
                    nc.gpsimd.partition_broadcast(bc, rrec)
                    nc.vector.tensor_mul(o2[64 * hh:64 * (hh + 1), :], poc[hh][0:64, :], bc)
                return o2

            def outproj_unit(qc, qt, o2l):
                row = (qc * 4 + qt) * 128
                pso = sps.tile([128, 1024], F32, tag="s", name="pso")
                for od in range(2):
                    osl = slice(od * 512, (od + 1) * 512)
                    for hp in range(2):
                        nc.tensor.matmul(
                            pso[:, osl],
                            o2l[hp][:, qt * 128:(qt + 1) * 128],
                            wo_sb[hp][:, osl],
                            start=(hp == 0),
                            stop=(hp == 1),
                        )
                ob = outp.tile([128, 1024], F32, tag="ob", name="ob")
                nc.vector.tensor_copy(ob, pso)
                nc.sync.dma_start(out=out[row:row + 128, :], in_=ob)

            # segments in (qc, hp) order; stream(s) runs scores+exp of segment
            # s while the PE retires AV of segment s-1 (p tiles buffer a full
            # segment). The last stream is coupled (AV follows its own exp) so
            # the tail is just normalize + the final out-projection.
            SEGS = [(qc, hp) for qc in range(QC) for hp in range(2)]
            NSEG = len(SEGS)
            p_of, po_of = {}, {}
            o2_all = {}

            # projection filler units, in first-needed order; stream(s) drains
            # the units segment s+1 needs, one full segment ahead of use.
            chunk_order = ([(3, c) for c in range(NCH)] + [(1, 0)]
                           + [(f, c) for c in range(1, NCH) for f in (0, 1)])
            for ft, c in chunk_order:
                work.append(lambda ft=ft, c=c: proj_chunk(ft, c))
            prereq = {1: 5}
            for s in range(2, NSEG):
                prereq[s] = 4 + s

            # head: k/q projections for segment 0 (sequential: only settled
            # stationaries are ever consumed)
            proj_chunk(2, 0)
            proj_chunk(0, 0)
            for c in range(1, NCH):
                proj_chunk(2, c)

            pend = emit_scores(0, 0)
            for sidx in range(NSEG):
                qc, hp = SEGS[sidx]
                coupled = (sidx == NSEG - 1)
                if sidx > 0:
                    # AV of the previous segment retires during this stream
                    po_of[sidx - 1] = [pops.tile([65, 512], F32, tag=f"po{hh}", name=f"po{hh}")
                                       for hh in range(2)]
                if coupled:
                    # last stream also retires its own AV; its accumulators
                    # live in the (now idle) qp ring banks
                    po_of[sidx] = [qps.tile([65, 512], F32, tag="qp", name=f"poz{hh}")
                                   for hh in range(2)]
                for i in range(KT):
                    ps_s = pend
                    nxt = (sidx, i + 1) if i + 1 < KT else (
                        (sidx + 1, 0) if sidx + 1 < NSEG else None)
                    if nxt is not None:
                        if nxt[0] != sidx and nxt[0] in prereq:
                            drain_to(prereq[nxt[0]])
                        pend = emit_scores(nxt[0], nxt[1])
                    emit_exp(sidx, i, ps_s)
                    if sidx == 0:
                        v_chunk(i)
                    else:
                        emit_av(sidx - 1, i)
                    if coupled:
                        emit_av(sidx, i)
                    elif i % 3 == 2 and i + 2 < KT:
                        drain(1)

                if sidx > 0:
                    s_done = sidx - 1
                    o2_all[s_done] = norm_pair(s_done)
                    qcd, hpd = SEGS[s_done]
                    if hpd == 1:
                        o2l = (o2_all.pop(2 * qcd), o2_all.pop(2 * qcd + 1))
                        for qt in range(4):
                            work.append(lambda qt=qt, qc=qcd, o2l=o2l: outproj_unit(qc, qt, o2l))
                if coupled:
                    o2_all[sidx] = norm_pair(sidx)
                    o2l = (o2_all.pop(sidx - 1), o2_all.pop(sidx))
                    for qt in range(4):
                        work.append(lambda qt=qt, qc=qc, o2l=o2l: outproj_unit(qc, qt, o2l))
            drain(len(work))

    nc.finalize()
    return nc


def rope_tables(n, d):
    """cos/sin tables in (d, n) layout, interleaved-repeat, theta=10000."""
    inv_freq = 1.0 / (10000.0 ** (np.arange(0, d, 2, dtype=np.float32) / d))
    ang = np.arange(n, dtype=np.float32)[:, None] * inv_freq[None, :]   # (n, d/2)
    cos = np.repeat(np.cos(ang), 2, axis=-1).T.copy()                    # (d, n)
    sin = np.repeat(np.sin(ang), 2, axis=-1).T.copy()
    return cos.astype(np.float32), sin.astype(np.float32)


def rot_matrix(d):
    """S with (S x)[2i] = -x[2i+1], (S x)[2i+1] = x[2i]."""
    S = np.zeros((d, d), dtype=np.float32)
    for i in range(d // 2):
        S[2 * i, 2 * i + 1] = -1.0
        S[2 * i + 1, 2 * i] = 1.0
    return S


def make_core_inputs(x, w_qkv, w_out, cfg: Cfg, core):
    n, dim = cfg.n_seq, cfg.dim
    b, g = core // TP, core % TP
    f0 = g * cfg.fpc
    inner = TP * cfg.fpc
    xT = np.ascontiguousarray(np.asarray(x)[b].T).astype(NPDT)
    w_qkv = np.asarray(w_qkv)
    wq = w_qkv[:, f0:f0 + cfg.fpc]
    wk = w_qkv[:, inner + f0:inner + f0 + cfg.fpc]
    wvv = np.ascontiguousarray(w_qkv[:, 2 * inner + f0:2 * inner + f0 + cfg.fpc]).astype(NPDT)
    wqk = np.ascontiguousarray(np.concatenate([wq, wk], axis=1)).astype(NPDT)
    wo = np.ascontiguousarray(np.asarray(w_out)[f0:f0 + cfg.fpc, :]).astype(NPDT)
    cos, sin = rope_tables(n, D)
    cosT = np.concatenate([cos, cos], axis=0).astype(NPDT)   # 2-head packed (128, n)
    sinT = np.concatenate([sin, sin], axis=0).astype(NPDT)
    S = rot_matrix(D)
    S128 = np.zeros((128, 128), dtype=np.float32)
    S128[0:64, 0:64] = S
    S128[64:128, 64:128] = S
    srot = np.ascontiguousarray(S128.T).astype(NPDT)
    onesv = np.ones((128, cfg.kt * HPC), dtype=NPDT)
    return {
        "xT": xT, "wqk": wqk, "wv": wvv, "wo": wo,
        "cosT": cosT, "sinT": sinT, "srot": srot, "onesv": onesv,
    }


_NC_CACHE = {}


def kernel(x, w_qkv, w_out, b_out):
    cfg = Cfg()
    key = (cfg.n_seq, cfg.dim)
    if key not in _NC_CACHE:
        _NC_CACHE[key] = build_nc(cfg)
    nc = _NC_CACHE[key]
    in_maps = [make_core_inputs(x, w_qkv, w_out, cfg, c) for c in range(N_CORES)]
    res = run_bass_kernel_spmd(nc, in_maps, core_ids=list(range(N_CORES)))
    partials = [r["out"] for r in res.results]
    out = np.empty((B, cfg.n_seq, cfg.dim), dtype=np.float32)
    for b in range(B):
        acc = partials[b * TP].astype(np.float32).copy()
        for g in range(1, TP):
            acc += partials[b * TP + g]
        out[b] = acc + np.asarray(b_out, dtype=np.float32)[None, :]
    return out
